# revision 37
# baseline (speedup 1.0000x reference)
"""Trainium2 Bass kernel for AspectNeighborAttention (gnn_message_passing).

Pure data-parallel over batch: 32 batches -> 8 NeuronCores x 4 batches.
All weights replicated, host-converted to bf16 and host-PRE-TRANSPOSED into
the chunk-major [128, KC, *] lhsT/rhs layouts the TensorEngine wants.

v2 redesign (from v1 at ~167us measured):
  * The zs GEMM is gone entirely: every consumer of zs is host-folded onto
    bertS directly (A2 = bertS @ (G0 Wz)^T, temp1 = bertS @ (WhZ Wz - I)^T
    which also folds the -bertS blend term, s_i/s_j = bertS @ (Wz^T wa_*)
    with the bz constants folded into ba / a bias row).  PE instruction
    count halves and the serial zsT dependency disappears.
  * dep is host-transposed to [i, e, j] (wa_e pre-folded, bf16; 1/wa_e
    folded into G1 rows so D' @ G1' == D @ G1 exactly):
      - s_e = sum_e dep': binary TT-add tree over the MIDDLE e axis.
        TensorReduce has NO DVE fast modes (1.04 ns/elem always) but
        TensorTensor has 2x_1p (0.52 ns/elem when every operand is 2-byte
        with innermost stride 1), so a 6-level tree (8064 elems) beats the
        single reduce (8192 elems) 2:1.
      - D-mult tmp = dep' * attn[i,j]-broadcast: the broadcast is over the
        middle axis so the innermost stride stays 1 -> 2x mode (v1's
        innermost-stride-0 broadcast forced 1x).
      - D-reduce over the innermost j axis: 7-level TT tree at 2x.
    DVE work per batch drops ~22us -> ~12us (cost-model validated).
  * Masking is additive and PE-folded: host sends madd^T (0 / -1e4), a
    maddT x Identity matmul accumulates it into the same PSUM tile as the
    s_j row broadcast, so score = lrelu(se + si_col + sjmadd) needs only
    2 DVE STTs; masked entries hit exp underflow -> exact 0 attn.  The
    row-max pass is dropped (scores are bounded ~+-8, exp is safe); sumex
    gets +1e-30 so all-masked rows yield attn=0 instead of NaN.
  * attn = ex * rec as a bf16 tensor_scalar (4x_2p mode, ~33ns).
  * GpSimd carries a balanced slice of each tree level + the e-tail of the
    D path; emission is software-pipelined (back(b-1) emitted after
    front(b)) so dT/G1/blend of batch b-1 never head-of-line-block batch
    b's PE/DVE front-end work.  PSUM: 3x p_big(2 banks) + 2x p_x(1) = 8.
"""

import sys

for _p in ("/opt/trn_rl_repo",):
    if _p not in sys.path:
        sys.path.insert(0, _p)

import os
import numpy as np
import ml_dtypes

import concourse.bass as bass
import concourse.bacc as bacc_mod
import concourse.mybir as mybir
import concourse.tile as tile
from concourse.masks import make_identity

B, L, H, E = 32, 128, 768, 64
NCORES = 8
PB = B // NCORES  # batches per core
KC = H // 128     # 6 k-chunks
F32 = mybir.dt.float32
BF16 = mybir.dt.bfloat16
AF = mybir.ActivationFunctionType
OP = mybir.AluOpType
AX = mybir.AxisListType
MASK_NEG = -10000.0

_CACHED = {}

CFG = dict(
    dep_bufs=int(os.environ.get("K_DEP_BUFS", 3)),
    tmpd_bufs=int(os.environ.get("K_TMPD_BUFS", 2)),
    spool_bufs=int(os.environ.get("K_SPOOL_BUFS", 3)),
    opool_bufs=int(os.environ.get("K_OPOOL_BUFS", 2)),
    px_bufs=int(os.environ.get("K_PX_BUFS", 2)),
    pbig_bufs=int(os.environ.get("K_PBIG_BUFS", 2)),
    edr=int(os.environ.get("K_EDR", 44)),
    edrl=int(os.environ.get("K_EDRL", 56)),  # last-batch DVE reduce rows   # D-reduce: e[0:edr) DVE, rest GpSimd
    actlrelu=int(os.environ.get("K_ACTLRELU", 1)),  # lrelu on ACT (Prelu)
)


def _build(debug=False):
    nc = bacc_mod.Bacc("TRN2", target_bir_lowering=False, debug=False,
                       num_devices=NCORES)

    bert = nc.dram_tensor("berts", [PB, L, H], F32, kind="ExternalInput")
    bertsT = nc.dram_tensor("bertsT", [PB, 128, KC, 128], BF16,
                            kind="ExternalInput")
    dept_d = nc.dram_tensor("dept", [PB, 128, E, 128], BF16,
                            kind="ExternalInput")
    maddT_d = nc.dram_tensor("maddT", [PB, 128, 128], BF16,
                             kind="ExternalInput")
    vrow = nc.dram_tensor("vrow", [1, PB, 128], BF16, kind="ExternalInput")
    g0wT_d = nc.dram_tensor("g0wT", [128, KC, H], BF16, kind="ExternalInput")
    m1T_d = nc.dram_tensor("m1T", [128, KC, H], BF16, kind="ExternalInput")
    g1_d = nc.dram_tensor("g1", [E, H], BF16, kind="ExternalInput")
    u2T_d = nc.dram_tensor("u2T", [128, KC, 2], BF16, kind="ExternalInput")
    browt = nc.dram_tensor("browt", [1, H], BF16, kind="ExternalInput")
    bat = nc.dram_tensor("bat", [1, 1], F32, kind="ExternalInput")
    out = nc.dram_tensor("out", [PB, L, H], F32, kind="ExternalOutput")

    dbg = {}
    if debug:
        for nm, shape, dt in [
            ("d_si", [1, 128], F32), ("d_sjb", [1, 128], F32),
            ("d_se", [128, L], BF16), ("d_sadd", [128, L], F32),
            ("d_attn", [128, L], BF16), ("d_dvec", [128, E], BF16),
            ("d_a2b", [128, H], BF16), ("d_upd", [128, 1], F32),
        ]:
            dbg[nm] = nc.dram_tensor(nm, shape, dt, kind="ExternalOutput")
    with tile.TileContext(nc) as tc:
        with nc.allow_low_precision("bf16 softmax/D path, 2e-2 rel-err gate"):
            _body(tc, nc, bert, bertsT, dept_d, maddT_d, vrow, g0wT_d, m1T_d,
                  g1_d, u2T_d, browt, bat, out, dbg)
    nc.compile()
    return nc


def _body(tc, nc, bert, bertsT, dept_d, maddT_d, vrow, g0wT_d, m1T_d,
          g1_d, u2T_d, browt, bat, out, dbg=None):
    def dump(name, ap):
        if dbg and name in dbg:
            nc.sync.dma_start(dbg[name][...], ap)
    import contextlib
    cfg = CFG
    EDR = cfg["edr"]
    ctx = contextlib.ExitStack()
    with ctx:
        wpool = ctx.enter_context(tc.tile_pool(name="weights", bufs=1))
        dpool = ctx.enter_context(
            tc.tile_pool(name="dep", bufs=cfg["dep_bufs"]))
        tpool = ctx.enter_context(
            tc.tile_pool(name="tmpd", bufs=cfg["tmpd_bufs"]))
        spool = ctx.enter_context(
            tc.tile_pool(name="small", bufs=cfg["spool_bufs"]))
        opool = ctx.enter_context(
            tc.tile_pool(name="outp", bufs=cfg["opool_bufs"]))
        # PSUM budget (8 banks): p_t [128,1024]f32 = 2 banks x2 bufs,
        # p_a [128,512]f32 = 1 bank x2, ptb [128,256]bf16 = 1 bank x2.
        p_apool = ctx.enter_context(
            tc.tile_pool(name="p_a", bufs=cfg["px_bufs"], space="PSUM"))
        p_tb = ctx.enter_context(
            tc.tile_pool(name="p_tb", bufs=cfg["px_bufs"], space="PSUM"))
        p_big = ctx.enter_context(
            tc.tile_pool(name="p_big", bufs=cfg["pbig_bufs"], space="PSUM"))

        # ---------------- input-batch prefetch (emitted FIRST so batch-0
        # dep isn't queued behind 2.4MB of weights) ----------------
        def prefetch(b):
            st = {}
            dept = dpool.tile([128, E, 128], BF16, tag="dept")
            nc.sync.dma_start(dept[:, 0:32, :], dept_d[b, :, 0:32, :])
            nc.sync.dma_start(dept[:, 32:64, :], dept_d[b, :, 32:64, :])
            bertST = spool.tile([128, KC, 128], BF16, tag="bertST")
            nc.sync.dma_start(bertST[:], bertsT[b, :, :, :])
            maddT = spool.tile([128, 128], BF16, tag="maddT")
            nc.sync.dma_start(maddT[:], maddT_d[b, :, :])
            bertS = spool.tile([128, H], F32, tag="bertS")
            nc.sync.dma_start(bertS[:], bert[b, :, :])
            st.update(bertS=bertS, dept=dept, bertST=bertST, maddT=maddT)
            return st

        st0 = prefetch(0)

        # ---------------- one-time setup (plain DMAs only).  The 2.4MB of
        # GEMM weights is deferred until after prefetch(1): nothing on the
        # DVE critical path needs them before mid-batch-0. ----------------
        g0wT = wpool.tile([128, KC, H], BF16, tag="g0wT")
        m1T = wpool.tile([128, KC, H], BF16, tag="m1T")
        g1 = wpool.tile([E, H], BF16, tag="g1")

        def load_big_weights():
            nc.sync.dma_start(g0wT[:], g0wT_d[...])
            nc.sync.dma_start(m1T[:], m1T_d[...])
            nc.sync.dma_start(g1[:], g1_d[...])

        u2T = wpool.tile([128, KC, 2], BF16, tag="u2T")
        nc.sync.dma_start(u2T[:], u2T_d[...])
        brow = wpool.tile([1, H], BF16, tag="brow")
        nc.sync.dma_start(brow[:], browt[:, :])
        bar = wpool.tile([1, 1], F32, tag="bar")
        nc.sync.dma_start(bar[:], bat[:, :])
        vrow4 = wpool.tile([1, PB, 128], BF16, tag="vrow4")
        nc.sync.dma_start(vrow4[:], vrow[:, :, :])

        ones_f = wpool.tile([1, 128], F32, tag="ones_f")
        nc.gpsimd.memset(ones_f[:], 1.0)
        ones_b = wpool.tile([1, 128], BF16, tag="ones_b")
        nc.gpsimd.memset(ones_b[:], 1.0)
        id_bf = wpool.tile([128, 128], BF16, tag="id_bf")
        make_identity(nc, id_bf[:])

        # -------- per-batch pipeline (2-stage software pipeline) --------
        # Iteration b emits: PE front(b) | se+score(b) | D-phase(b-1) |
        # softmax tail(b) | finish(b-1).  Batch b's cross-engine softmax
        # latency hides entirely under batch b-1's D-phase on the DVE.
        def pe_front(b, st):
            bertST, maddT = st["bertST"], st["maddT"]
            # ---- A2 = bertS @ (G0 Wz)^T (two chunks, 1-bank ring) ----
            a2b = spool.tile([128, H], BF16, tag="a2b")
            for ns in (slice(0, 512), slice(512, H)):
                p_a = p_apool.tile([128, 512], F32, tag="p_a")
                w = ns.stop - ns.start
                for kc in range(KC):
                    nc.tensor.matmul(p_a[:, 0:w], bertST[:, kc, :],
                                     g0wT[:, kc, ns],
                                     start=(kc == 0), stop=(kc == KC - 1))
                nc.scalar.copy(a2b[:, ns], p_a[:, 0:w])
            if b == 0:
                dump("d_a2b", a2b[:])

            # p_t [128,1024] = 2 banks: [0:768] temp accum; [768:896] s_i row
            # then (WAR) sj+madd bcast; [896:1024] s_j row then (WAR) si col
            # at 1023 and upd col at 1022.
            p_t = p_big.tile([128, 1024], F32, tag="p_big")

            # ---- s_i / s_j rows (m=1 each; DVE can't read partition 1) ----
            for kc in range(KC):
                nc.tensor.matmul(p_t[0:1, 768:896], u2T[:, kc, 0:1],
                                 bertST[:, kc, :],
                                 start=(kc == 0), stop=(kc == KC - 1))
            for kc in range(KC):
                nc.tensor.matmul(p_t[0:1, 896:1024], u2T[:, kc, 1:2],
                                 bertST[:, kc, :],
                                 start=(kc == 0), stop=(kc == KC - 1))
            si_row = spool.tile([1, 128], F32, tag="si_row")
            nc.scalar.copy(si_row[:], p_t[0:1, 768:896])
            sjb = spool.tile([1, 128], F32, tag="sjb")
            nc.vector.tensor_scalar(sjb[:], p_t[0:1, 896:1024], bar[0:1, 0:1],
                                    None, op0=OP.add)
            # sj row bcast + additive mask (WAR over s_i region), si col
            nc.tensor.matmul(p_t[:, 768:896], maddT[:], id_bf[:],
                             start=True, stop=False)
            nc.tensor.matmul(p_t[:, 768:896], ones_f[:], sjb[:],
                             start=False, stop=True)
            nc.tensor.matmul(p_t[:, 1023:1024], si_row[:], ones_f[0:1, 0:1],
                             start=True, stop=True)
            if b == 0:
                dump("d_si", si_row[:])
                dump("d_sjb", sjb[:])

            # ---- temp1 = bertS @ (WhZ Wz - I)^T + brow ----
            for ns in (slice(0, 512), slice(512, H)):
                for kc in range(KC):
                    nc.tensor.matmul(p_t[:, ns], bertST[:, kc, :],
                                     m1T[:, kc, ns],
                                     start=(kc == 0), stop=False)
                nc.tensor.matmul(p_t[:, ns], ones_b[:], brow[0:1, ns],
                                 start=False, stop=False)
            st.update(a2b=a2b, p_t=p_t)

        def score_phase(b, st):
            dept, p_t = st["dept"], st["p_t"]
            # ---- s_e: 6-level TT tree over the middle e axis, ALL on DVE.
            # Every level keeps FULL-j (256B) innermost runs -> 2x packed
            # mode.  GpSimd is kept IDLE here: concurrent Pool traffic
            # degrades DVE 2x ops to ~1.3-1.8 ns/elem (measured). ----
            seA = tpool.tile([128, 32, 128], BF16, tag="seA")
            seB = tpool.tile([128, 16, 128], BF16, tag="seB")
            nc.vector.tensor_tensor(seA[:, 0:16, :], dept[:, 0:16, :],
                                    dept[:, 16:32, :], op=OP.add)
            nc.vector.tensor_tensor(seA[:, 16:32, :], dept[:, 32:48, :],
                                    dept[:, 48:64, :], op=OP.add)
            nc.vector.tensor_tensor(seB[:, :, :], seA[:, 0:16, :],
                                    seA[:, 16:32, :], op=OP.add)
            nc.vector.tensor_tensor(seA[:, 0:8, :], seB[:, 0:8, :],
                                    seB[:, 8:16, :], op=OP.add)
            nc.vector.tensor_tensor(seB[:, 0:4, :], seA[:, 0:4, :],
                                    seA[:, 4:8, :], op=OP.add)
            nc.vector.tensor_tensor(seA[:, 0:2, :], seB[:, 0:2, :],
                                    seB[:, 2:4, :], op=OP.add)
            sef = spool.tile([128, 128], BF16, tag="sef")
            nc.vector.tensor_tensor(sef[:], seA[:, 0, :], seA[:, 1, :],
                                    op=OP.add)
            if b == 0:
                dump("d_se", sef[:])

            # ---- score = lrelu(se + si + sj + madd); no rowmax (scores are
            # bounded ~+-8 so exp is overflow-safe; masked lanes underflow
            # to exact 0) ----
            sadd = spool.tile([128, L], F32, tag="sadd")
            nc.vector.scalar_tensor_tensor(
                sadd[:], sef[:], p_t[:, 1023:1024], p_t[:, 768:896],
                op0=OP.add, op1=OP.add)
            score = spool.tile([128, L], F32, tag="score")
            if cfg["actlrelu"]:
                # parametric_relu lives in the same ACT table set as Exp/Copy
                nc.scalar.activation(score[:], sadd[:], AF.Prelu, bias=0.0,
                                     scale=1.0, alpha=0.01)
            else:
                nc.vector.scalar_tensor_tensor(
                    score[:], sadd[:], 0.01, sadd[:], op0=OP.mult, op1=OP.max)
            if b == 0:
                dump("d_sadd", score[:])
            ex = spool.tile([128, L], BF16, tag="ex")
            sumex = spool.tile([128, 1], F32, tag="sumex")
            nc.scalar.activation(ex[:], score[:], AF.Exp, bias=0.0,
                                 scale=1.0, accum_out=sumex[:])
            st.update(ex=ex, sumex=sumex)

        def softmax_tail(b, st):
            sume = spool.tile([128, 1], F32, tag="sume")
            nc.vector.tensor_scalar(sume[:], st["sumex"][:], 1e-30, None,
                                    op0=OP.add)
            rec = spool.tile([128, 1], F32, tag="rec")
            nc.vector.reciprocal(rec[:], sume[:])
            # attn = ex * rec on ACT (Copy with per-partition scale)
            attnb = spool.tile([128, L], BF16, tag="attnb")
            nc.scalar.activation(attnb[:], st["ex"][:], AF.Copy, bias=0.0,
                                 scale=rec[0:128, 0:1])
            if b == 0:
                dump("d_attn", attnb[:])
            # attn^T via PE (bf16 PSUM ring shared with the dT transpose)
            ptb = p_tb.tile([128, 256], BF16, tag="p_tb")
            nc.tensor.transpose(ptb[:, 0:128], attnb[:], id_bf[:])
            attnT = spool.tile([128, 128], BF16, tag="attnT")
            nc.scalar.copy(attnT[:], ptb[:, 0:128])
            # ---- attn @ A2 into p_t ----
            for ns in (slice(0, 512), slice(512, H)):
                nc.tensor.matmul(st["p_t"][:, ns], attnT[:],
                                 st["a2b"][:, ns], start=False, stop=False)
            st.update(attnb=attnb, ptb=ptb)

        def dphase(b, st, EDR):
            dept, attnb = st["dept"], st["attnb"]
            # ---- D-mult all on DVE at 2x, while GpSimd is idle (Pool
            # concurrency would halve the 2x rate); head first, tail second,
            # so the GpSimd tree only ever overlaps the 1x DVE reduce ----
            tmpD = tpool.tile([128, E, 128], BF16, tag="tmpD")
            nc.vector.tensor_tensor(
                tmpD[:, 0:EDR, :], dept[:, 0:EDR, :],
                attnb[:].unsqueeze(1).broadcast_to([128, EDR, 128]),
                op=OP.mult)
            nc.vector.tensor_tensor(
                tmpD[:, EDR:E, :], dept[:, EDR:E, :],
                attnb[:].unsqueeze(1).broadcast_to([128, E - EDR, 128]),
                op=OP.mult)
            tDs = tpool.tile([128, E - EDR, 64], BF16, tag="tDs")
            dvb = spool.tile([128, E], BF16, tag="dvb")
            nc.gpsimd.tensor_tensor(tDs[:, :, :], tmpD[:, EDR:E, 0:64],
                                    tmpD[:, EDR:E, 64:128], op=OP.add)
            nc.vector.tensor_reduce(dvb[:, 0:EDR], tmpD[:, 0:EDR, :],
                                    axis=AX.X, op=OP.add)
            nc.gpsimd.tensor_tensor(tmpD[:, EDR:E, 0:32], tDs[:, :, 0:32],
                                    tDs[:, :, 32:64], op=OP.add)
            nc.gpsimd.tensor_tensor(tDs[:, :, 0:16], tmpD[:, EDR:E, 0:16],
                                    tmpD[:, EDR:E, 16:32], op=OP.add)
            nc.gpsimd.tensor_tensor(tmpD[:, EDR:E, 0:8], tDs[:, :, 0:8],
                                    tDs[:, :, 8:16], op=OP.add)
            nc.gpsimd.tensor_tensor(tDs[:, :, 0:4], tmpD[:, EDR:E, 0:4],
                                    tmpD[:, EDR:E, 4:8], op=OP.add)
            nc.gpsimd.tensor_tensor(tmpD[:, EDR:E, 0:2], tDs[:, :, 0:2],
                                    tDs[:, :, 2:4], op=OP.add)
            nc.gpsimd.tensor_tensor(dvb[:, EDR:E], tmpD[:, EDR:E, 0:1],
                                    tmpD[:, EDR:E, 1:2], op=OP.add)
            if b == 0:
                dump("d_dvec", dvb[:])
            st.update(dvb=dvb)

        def finish(b, st):
            p_t = st["p_t"]
            nc.tensor.transpose(st["ptb"][0:E, 128:256], st["dvb"][:],
                                id_bf[:])
            dT = spool.tile([E, 128], BF16, tag="dT")
            nc.scalar.copy(dT[:], st["ptb"][0:E, 128:256])
            for ns in (slice(0, 512), slice(512, H)):
                nc.tensor.matmul(p_t[:, ns], dT[:], g1[:, ns],
                                 start=False, stop=True)
            nc.tensor.matmul(p_t[:, 1022:1023], vrow4[0:1, b, :],
                             ones_b[0:1, 0:1], start=True, stop=True)
            if b == 0:
                dump("d_upd", p_t[:, 1022:1023])
            outt = opool.tile([128, H], F32, tag="outt")
            nc.vector.scalar_tensor_tensor(
                outt[:], p_t[:, 0:H], p_t[:, 1022:1023], st["bertS"][:],
                op0=OP.mult, op1=OP.add)
            # store via GpSimd SWDGE: its descriptors spread across all 16
            # DMA engines, while HWDGE (sync/scalar) dma_starts serialize on
            # ONE engine (~25GB/s) -- that cost a ~20us kernel-tail backlog
            nc.sync.dma_start(out[b, 1:32, :], outt[0:31, :])
            nc.gpsimd.dma_start(out[b, 32:80, :], outt[31:79, :])
            nc.gpsimd.dma_start(out[b, 80:128, :], outt[79:127, :])
            nc.sync.dma_start(out[b, 0:1, :], outt[127:128, :])

        sts = {0: st0}
        for b in range(PB):
            # prefetch first so dep(b+1) is never queued behind stores
            if b + 1 < PB:
                sts[b + 1] = prefetch(b + 1)
            if b == 0:
                load_big_weights()
            pe_front(b, sts[b])
            score_phase(b, sts[b])
            if b >= 1:
                dphase(b - 1, sts[b - 1], EDR)
            softmax_tail(b, sts[b])
            if b >= 1:
                finish(b - 1, sts.pop(b - 1))
        # final batch: DVE-heavy reduce split (DVE is otherwise idle in the
        # tail, and the GpSimd tree would pace the whole epilogue)
        dphase(PB - 1, sts[PB - 1], cfg["edrl"])
        finish(PB - 1, sts.pop(PB - 1))


def _get_nc():
    if "nc" not in _CACHED:
        _CACHED["nc"] = _build(debug=bool(_CACHED.get("debug")))
    return _CACHED["nc"]


def _chunkT(w):
    """W [rows, K] -> W^T chunk-major [128, K//128, rows] (lhsT layout)."""
    rows, k = w.shape
    return np.ascontiguousarray(
        w.T.reshape(k // 128, 128, rows).transpose(1, 0, 2))


def _prep_in_maps(bert_hidden_states, dep_type_adj, deprel_adj,
                  asp_start, asp_end, Wz, bz, wa, ba, Wf, Wh):
    bf = ml_dtypes.bfloat16
    bert = np.ascontiguousarray(np.asarray(bert_hidden_states, np.float32))
    wa_f = np.asarray(wa, np.float32)
    wa_i, wa_j, wae_f = wa_f[:H], wa_f[H:2 * H], wa_f[2 * H:]
    wae_safe = np.where(wae_f == 0.0, 1.0, wae_f)
    # dep': wa_e folded in, transposed to [b, i, e, j]
    depW = np.asarray(dep_type_adj, np.float32) * wae_f[None, None, None, :]
    dept = np.ascontiguousarray(depW.transpose(0, 1, 3, 2)).astype(bf)
    adjn = np.asarray(deprel_adj) > 0
    madd = np.where(adjn, np.float32(0.0), np.float32(MASK_NEG))
    maddT = np.ascontiguousarray(madd.transpose(0, 2, 1)).astype(bf)
    # bertS^T chunk-major per batch: rows shifted by one (the z-roll)
    bs = np.ascontiguousarray(np.roll(bert, -1, axis=1))
    bertsT = np.ascontiguousarray(
        bs.transpose(0, 2, 1).reshape(B, KC, 128, L).transpose(0, 2, 1, 3)
    ).astype(bf)
    pos = np.arange(L, dtype=np.float32)
    s_ = np.asarray(asp_start).astype(np.float32)[:, None]
    e_ = np.asarray(asp_end).astype(np.float32)[:, None]
    vrow_full = (((pos[None, :] >= s_) & (pos[None, :] <= e_))
                 & adjn.any(-1)).astype(bf)

    Wz = np.asarray(Wz, np.float32)
    bz_f = np.asarray(bz, np.float32)
    ba_f = np.float32(np.asarray(ba, np.float32))
    Wf = np.asarray(Wf, np.float32)
    Wh = np.asarray(Wh, np.float32)
    WfZ, WfE = Wf[:, :H], Wf[:, H:]
    WhN, WhZ = Wh[:, :H], Wh[:, H:]
    G0 = WhN @ WfZ
    g0wT = _chunkT(G0 @ Wz).astype(bf)
    m1T = _chunkT(WhZ @ Wz - np.eye(H, dtype=np.float32)).astype(bf)
    g1 = np.ascontiguousarray(
        (WhN @ WfE).T / wae_safe[:, None]).astype(bf)
    u2 = np.stack([Wz.T @ wa_i, Wz.T @ wa_j], axis=0)  # [2, H]
    u2T = _chunkT(u2).astype(bf)
    brow = (WhZ @ bz_f + G0 @ bz_f)[None, :].astype(bf)
    bab = np.float32(ba_f + wa_i @ bz_f + wa_j @ bz_f).reshape(1, 1)

    in_maps = []
    for c in range(NCORES):
        s = slice(c * PB, (c + 1) * PB)
        in_maps.append(dict(
            berts=bs[s], bertsT=np.ascontiguousarray(bertsT[s]),
            dept=dept[s], maddT=maddT[s],
            vrow=np.ascontiguousarray(vrow_full[s][None, :, :]),
            g0wT=g0wT, m1T=m1T, g1=g1, u2T=u2T,
            browt=brow, bat=bab,
        ))
    return in_maps


def kernel(bert_hidden_states, dep_type_adj, deprel_adj, asp_start, asp_end,
           Wz, bz, wa, ba, Wf, Wh):
    from concourse.bass_utils import run_bass_kernel_spmd

    in_maps = _prep_in_maps(bert_hidden_states, dep_type_adj, deprel_adj,
                            asp_start, asp_end, Wz, bz, wa, ba, Wf, Wh)
    nc = _get_nc()
    res = run_bass_kernel_spmd(nc, in_maps, core_ids=list(range(NCORES)),
                               trace=bool(_CACHED.get("trace")),
                               tmpdir=_CACHED.get("trace_tmpdir"))
    _CACHED["last_results"] = res
    outs = [res.results[c]["out"] for c in range(NCORES)]
    return np.concatenate(outs, axis=0).astype(np.float32)


# revision 38
# speedup vs baseline: 1.0248x; 1.0248x over previous
"""Trainium2 Bass kernel for AspectNeighborAttention (gnn_message_passing).

Pure data-parallel over batch: 32 batches -> 8 NeuronCores x 4 batches.
All weights replicated, host-converted to bf16 and host-PRE-TRANSPOSED into
the chunk-major [128, KC, *] lhsT/rhs layouts the TensorEngine wants.

v2 redesign (from v1 at ~167us measured):
  * The zs GEMM is gone entirely: every consumer of zs is host-folded onto
    bertS directly (A2 = bertS @ (G0 Wz)^T, temp1 = bertS @ (WhZ Wz - I)^T
    which also folds the -bertS blend term, s_i/s_j = bertS @ (Wz^T wa_*)
    with the bz constants folded into ba / a bias row).  PE instruction
    count halves and the serial zsT dependency disappears.
  * dep is host-transposed to [i, e, j] (wa_e pre-folded, bf16; 1/wa_e
    folded into G1 rows so D' @ G1' == D @ G1 exactly):
      - s_e = sum_e dep': binary TT-add tree over the MIDDLE e axis.
        TensorReduce has NO DVE fast modes (1.04 ns/elem always) but
        TensorTensor has 2x_1p (0.52 ns/elem when every operand is 2-byte
        with innermost stride 1), so a 6-level tree (8064 elems) beats the
        single reduce (8192 elems) 2:1.
      - D-mult tmp = dep' * attn[i,j]-broadcast: the broadcast is over the
        middle axis so the innermost stride stays 1 -> 2x mode (v1's
        innermost-stride-0 broadcast forced 1x).
      - D-reduce over the innermost j axis: 7-level TT tree at 2x.
    DVE work per batch drops ~22us -> ~12us (cost-model validated).
  * Masking is additive and PE-folded: host sends madd^T (0 / -1e4), a
    maddT x Identity matmul accumulates it into the same PSUM tile as the
    s_j row broadcast, so score = lrelu(se + si_col + sjmadd) needs only
    2 DVE STTs; masked entries hit exp underflow -> exact 0 attn.  The
    row-max pass is dropped (scores are bounded ~+-8, exp is safe); sumex
    gets +1e-30 so all-masked rows yield attn=0 instead of NaN.
  * attn = ex * rec as a bf16 tensor_scalar (4x_2p mode, ~33ns).
  * GpSimd carries a balanced slice of each tree level + the e-tail of the
    D path; emission is software-pipelined (back(b-1) emitted after
    front(b)) so dT/G1/blend of batch b-1 never head-of-line-block batch
    b's PE/DVE front-end work.  PSUM: 3x p_big(2 banks) + 2x p_x(1) = 8.
"""

import sys

for _p in ("/opt/trn_rl_repo",):
    if _p not in sys.path:
        sys.path.insert(0, _p)

import os
import numpy as np
import ml_dtypes

import concourse.bass as bass
import concourse.bacc as bacc_mod
import concourse.mybir as mybir
import concourse.tile as tile
from concourse.masks import make_identity

B, L, H, E = 32, 128, 768, 64
NCORES = 8
PB = B // NCORES  # batches per core
KC = H // 128     # 6 k-chunks
F32 = mybir.dt.float32
BF16 = mybir.dt.bfloat16
AF = mybir.ActivationFunctionType
OP = mybir.AluOpType
AX = mybir.AxisListType
MASK_NEG = -10000.0

_CACHED = {}

CFG = dict(
    dep_bufs=int(os.environ.get("K_DEP_BUFS", 3)),
    tmpd_bufs=int(os.environ.get("K_TMPD_BUFS", 2)),
    spool_bufs=int(os.environ.get("K_SPOOL_BUFS", 3)),
    opool_bufs=int(os.environ.get("K_OPOOL_BUFS", 2)),
    px_bufs=int(os.environ.get("K_PX_BUFS", 2)),
    pbig_bufs=int(os.environ.get("K_PBIG_BUFS", 2)),
    edr=int(os.environ.get("K_EDR", 44)),
    edrl=int(os.environ.get("K_EDRL", 56)),  # last-batch DVE reduce rows   # D-reduce: e[0:edr) DVE, rest GpSimd
    actlrelu=int(os.environ.get("K_ACTLRELU", 1)),  # lrelu on ACT (Prelu)
)


def _build(debug=False):
    nc = bacc_mod.Bacc("TRN2", target_bir_lowering=False, debug=False,
                       num_devices=NCORES)

    bert = nc.dram_tensor("berts", [PB, L, H], F32, kind="ExternalInput")
    bertsT = nc.dram_tensor("bertsT", [PB, 128, KC, 128], BF16,
                            kind="ExternalInput")
    dept_d = nc.dram_tensor("dept", [PB, 128, E, 128], BF16,
                            kind="ExternalInput")
    maddT_d = nc.dram_tensor("maddT", [PB, 128, 128], BF16,
                             kind="ExternalInput")
    vrow = nc.dram_tensor("vrow", [1, PB, 128], BF16, kind="ExternalInput")
    g0wT_d = nc.dram_tensor("g0wT", [128, KC, H], BF16, kind="ExternalInput")
    m1T_d = nc.dram_tensor("m1T", [128, KC, H], BF16, kind="ExternalInput")
    g1_d = nc.dram_tensor("g1", [E, H], BF16, kind="ExternalInput")
    u2T_d = nc.dram_tensor("u2T", [128, KC, 2], BF16, kind="ExternalInput")
    browt = nc.dram_tensor("browt", [1, H], BF16, kind="ExternalInput")
    bat = nc.dram_tensor("bat", [1, 1], F32, kind="ExternalInput")
    out = nc.dram_tensor("out", [PB, L, H], F32, kind="ExternalOutput")

    dbg = {}
    if debug:
        for nm, shape, dt in [
            ("d_si", [1, 128], F32), ("d_sjb", [1, 128], F32),
            ("d_se", [128, L], BF16), ("d_sadd", [128, L], F32),
            ("d_attn", [128, L], BF16), ("d_dvec", [128, E], BF16),
            ("d_a2b", [128, H], BF16), ("d_upd", [128, 1], F32),
        ]:
            dbg[nm] = nc.dram_tensor(nm, shape, dt, kind="ExternalOutput")
    with tile.TileContext(nc) as tc:
        with nc.allow_low_precision("bf16 softmax/D path, 2e-2 rel-err gate"):
            _body(tc, nc, bert, bertsT, dept_d, maddT_d, vrow, g0wT_d, m1T_d,
                  g1_d, u2T_d, browt, bat, out, dbg)
    nc.compile()
    return nc


def _body(tc, nc, bert, bertsT, dept_d, maddT_d, vrow, g0wT_d, m1T_d,
          g1_d, u2T_d, browt, bat, out, dbg=None):
    def dump(name, ap):
        if dbg and name in dbg:
            nc.sync.dma_start(dbg[name][...], ap)
    import contextlib
    cfg = CFG
    EDR = cfg["edr"]
    ctx = contextlib.ExitStack()
    with ctx:
        wpool = ctx.enter_context(tc.tile_pool(name="weights", bufs=1))
        dpool = ctx.enter_context(
            tc.tile_pool(name="dep", bufs=cfg["dep_bufs"]))
        tpool = ctx.enter_context(
            tc.tile_pool(name="tmpd", bufs=cfg["tmpd_bufs"]))
        spool = ctx.enter_context(
            tc.tile_pool(name="small", bufs=cfg["spool_bufs"]))
        opool = ctx.enter_context(
            tc.tile_pool(name="outp", bufs=cfg["opool_bufs"]))
        # PSUM budget (8 banks): p_t [128,1024]f32 = 2 banks x2 bufs,
        # p_a [128,512]f32 = 1 bank x2, ptb [128,256]bf16 = 1 bank x2.
        p_apool = ctx.enter_context(
            tc.tile_pool(name="p_a", bufs=cfg["px_bufs"], space="PSUM"))
        p_tb = ctx.enter_context(
            tc.tile_pool(name="p_tb", bufs=cfg["px_bufs"], space="PSUM"))
        p_big = ctx.enter_context(
            tc.tile_pool(name="p_big", bufs=cfg["pbig_bufs"], space="PSUM"))

        # ---------------- input-batch prefetch (emitted FIRST so batch-0
        # dep isn't queued behind 2.4MB of weights) ----------------
        def prefetch(b):
            st = {}
            dept = dpool.tile([128, E, 128], BF16, tag="dept")
            nc.sync.dma_start(dept[:, 0:32, :], dept_d[b, :, 0:32, :])
            nc.sync.dma_start(dept[:, 32:64, :], dept_d[b, :, 32:64, :])
            bertST = spool.tile([128, KC, 128], BF16, tag="bertST")
            nc.sync.dma_start(bertST[:], bertsT[b, :, :, :])
            maddT = spool.tile([128, 128], BF16, tag="maddT")
            nc.sync.dma_start(maddT[:], maddT_d[b, :, :])
            bertS = spool.tile([128, H], F32, tag="bertS")
            nc.sync.dma_start(bertS[:], bert[b, :, :])
            st.update(bertS=bertS, dept=dept, bertST=bertST, maddT=maddT)
            return st

        st0 = prefetch(0)

        # ---------------- one-time setup (plain DMAs only).  The 2.4MB of
        # GEMM weights is deferred until after prefetch(1): nothing on the
        # DVE critical path needs them before mid-batch-0. ----------------
        g0wT = wpool.tile([128, KC, H], BF16, tag="g0wT")
        m1T = wpool.tile([128, KC, H], BF16, tag="m1T")
        g1 = wpool.tile([E, H], BF16, tag="g1")

        def load_big_weights():
            nc.sync.dma_start(g0wT[:], g0wT_d[...])
            nc.sync.dma_start(m1T[:], m1T_d[...])
            nc.sync.dma_start(g1[:], g1_d[...])

        u2T = wpool.tile([128, KC, 2], BF16, tag="u2T")
        nc.sync.dma_start(u2T[:], u2T_d[...])
        brow = wpool.tile([1, H], BF16, tag="brow")
        nc.sync.dma_start(brow[:], browt[:, :])
        bar = wpool.tile([1, 1], F32, tag="bar")
        nc.sync.dma_start(bar[:], bat[:, :])
        vrow4 = wpool.tile([1, PB, 128], BF16, tag="vrow4")
        nc.sync.dma_start(vrow4[:], vrow[:, :, :])

        ones_f = wpool.tile([1, 128], F32, tag="ones_f")
        nc.gpsimd.memset(ones_f[:], 1.0)
        ones_b = wpool.tile([1, 128], BF16, tag="ones_b")
        nc.gpsimd.memset(ones_b[:], 1.0)
        id_bf = wpool.tile([128, 128], BF16, tag="id_bf")
        make_identity(nc, id_bf[:])

        # -------- per-batch pipeline (2-stage software pipeline) --------
        # Iteration b emits: PE front(b) | se+score(b) | D-phase(b-1) |
        # softmax tail(b) | finish(b-1).  Batch b's cross-engine softmax
        # latency hides entirely under batch b-1's D-phase on the DVE.
        def pe_front(b, st):
            bertST, maddT = st["bertST"], st["maddT"]
            # ---- A2 = bertS @ (G0 Wz)^T (two chunks, 1-bank ring) ----
            a2b = spool.tile([128, H], BF16, tag="a2b")
            for ns in (slice(0, 512), slice(512, H)):
                p_a = p_apool.tile([128, 512], F32, tag="p_a")
                w = ns.stop - ns.start
                for kc in range(KC):
                    nc.tensor.matmul(p_a[:, 0:w], bertST[:, kc, :],
                                     g0wT[:, kc, ns],
                                     start=(kc == 0), stop=(kc == KC - 1))
                nc.scalar.copy(a2b[:, ns], p_a[:, 0:w])
            if b == 0:
                dump("d_a2b", a2b[:])

            # p_t [128,1024] = 2 banks: [0:768] temp accum; [768:896] s_i row
            # then (WAR) sj+madd bcast; [896:1024] s_j row then (WAR) si col
            # at 1023 and upd col at 1022.
            p_t = p_big.tile([128, 1024], F32, tag="p_big")

            # ---- s_i / s_j rows (m=1 each; DVE can't read partition 1) ----
            for kc in range(KC):
                nc.tensor.matmul(p_t[0:1, 768:896], u2T[:, kc, 0:1],
                                 bertST[:, kc, :],
                                 start=(kc == 0), stop=(kc == KC - 1))
            for kc in range(KC):
                nc.tensor.matmul(p_t[0:1, 896:1024], u2T[:, kc, 1:2],
                                 bertST[:, kc, :],
                                 start=(kc == 0), stop=(kc == KC - 1))
            si_row = spool.tile([1, 128], F32, tag="si_row")
            nc.scalar.copy(si_row[:], p_t[0:1, 768:896])
            sjb = spool.tile([1, 128], F32, tag="sjb")
            nc.vector.tensor_scalar(sjb[:], p_t[0:1, 896:1024], bar[0:1, 0:1],
                                    None, op0=OP.add)
            # sj row bcast + additive mask (WAR over s_i region), si col
            nc.tensor.matmul(p_t[:, 768:896], maddT[:], id_bf[:],
                             start=True, stop=False)
            nc.tensor.matmul(p_t[:, 768:896], ones_f[:], sjb[:],
                             start=False, stop=True)
            nc.tensor.matmul(p_t[:, 1023:1024], si_row[:], ones_f[0:1, 0:1],
                             start=True, stop=True)
            if b == 0:
                dump("d_si", si_row[:])
                dump("d_sjb", sjb[:])

            # ---- temp1 = bertS @ (WhZ Wz - I)^T + brow ----
            for ns in (slice(0, 512), slice(512, H)):
                for kc in range(KC):
                    nc.tensor.matmul(p_t[:, ns], bertST[:, kc, :],
                                     m1T[:, kc, ns],
                                     start=(kc == 0), stop=False)
                nc.tensor.matmul(p_t[:, ns], ones_b[:], brow[0:1, ns],
                                 start=False, stop=False)
            st.update(a2b=a2b, p_t=p_t)

        def score_phase(b, st):
            dept, p_t = st["dept"], st["p_t"]
            # ---- s_e: 6-level TT tree over the middle e axis, ALL on DVE.
            # Every level keeps FULL-j (256B) innermost runs -> 2x packed
            # mode.  GpSimd is kept IDLE here: concurrent Pool traffic
            # degrades DVE 2x ops to ~1.3-1.8 ns/elem (measured). ----
            seA = tpool.tile([128, 32, 128], BF16, tag="seA")
            seB = tpool.tile([128, 16, 128], BF16, tag="seB")
            nc.vector.tensor_tensor(seA[:, 0:16, :], dept[:, 0:16, :],
                                    dept[:, 16:32, :], op=OP.add)
            nc.vector.tensor_tensor(seA[:, 16:32, :], dept[:, 32:48, :],
                                    dept[:, 48:64, :], op=OP.add)
            nc.vector.tensor_tensor(seB[:, :, :], seA[:, 0:16, :],
                                    seA[:, 16:32, :], op=OP.add)
            nc.vector.tensor_tensor(seA[:, 0:8, :], seB[:, 0:8, :],
                                    seB[:, 8:16, :], op=OP.add)
            nc.vector.tensor_tensor(seB[:, 0:4, :], seA[:, 0:4, :],
                                    seA[:, 4:8, :], op=OP.add)
            nc.vector.tensor_tensor(seA[:, 0:2, :], seB[:, 0:2, :],
                                    seB[:, 2:4, :], op=OP.add)
            sef = spool.tile([128, 128], BF16, tag="sef")
            nc.vector.tensor_tensor(sef[:], seA[:, 0, :], seA[:, 1, :],
                                    op=OP.add)
            if b == 0:
                dump("d_se", sef[:])

            # ---- score = lrelu(se + si + sj + madd); no rowmax (scores are
            # bounded ~+-8 so exp is overflow-safe; masked lanes underflow
            # to exact 0) ----
            sadd = spool.tile([128, L], F32, tag="sadd")
            nc.vector.scalar_tensor_tensor(
                sadd[:], sef[:], p_t[:, 1023:1024], p_t[:, 768:896],
                op0=OP.add, op1=OP.add)
            score = spool.tile([128, L], F32, tag="score")
            if cfg["actlrelu"]:
                # parametric_relu lives in the same ACT table set as Exp/Copy
                nc.scalar.activation(score[:], sadd[:], AF.Prelu, bias=0.0,
                                     scale=1.0, alpha=0.01)
            else:
                nc.vector.scalar_tensor_tensor(
                    score[:], sadd[:], 0.01, sadd[:], op0=OP.mult, op1=OP.max)
            if b == 0:
                dump("d_sadd", score[:])
            ex = spool.tile([128, L], BF16, tag="ex")
            sumex = spool.tile([128, 1], F32, tag="sumex")
            nc.scalar.activation(ex[:], score[:], AF.Exp, bias=0.0,
                                 scale=1.0, accum_out=sumex[:])
            st.update(ex=ex, sumex=sumex)

        def softmax_tail(b, st):
            sume = spool.tile([128, 1], F32, tag="sume")
            nc.vector.tensor_scalar(sume[:], st["sumex"][:], 1e-30, None,
                                    op0=OP.add)
            rec = spool.tile([128, 1], F32, tag="rec")
            nc.vector.reciprocal(rec[:], sume[:])
            # attn = ex * rec on ACT (Copy with per-partition scale)
            attnb = spool.tile([128, L], BF16, tag="attnb")
            nc.scalar.activation(attnb[:], st["ex"][:], AF.Copy, bias=0.0,
                                 scale=rec[0:128, 0:1])
            if b == 0:
                dump("d_attn", attnb[:])
            # attn^T via PE (bf16 PSUM ring shared with the dT transpose)
            ptb = p_tb.tile([128, 256], BF16, tag="p_tb")
            nc.tensor.transpose(ptb[:, 0:128], attnb[:], id_bf[:])
            attnT = spool.tile([128, 128], BF16, tag="attnT")
            nc.scalar.copy(attnT[:], ptb[:, 0:128])
            # ---- attn @ A2 into p_t ----
            for ns in (slice(0, 512), slice(512, H)):
                nc.tensor.matmul(st["p_t"][:, ns], attnT[:],
                                 st["a2b"][:, ns], start=False, stop=False)
            st.update(attnb=attnb, ptb=ptb)

        def dphase(b, st, EDR):
            dept, attnb = st["dept"], st["attnb"]
            # ---- D-mult all on DVE at 2x, while GpSimd is idle (Pool
            # concurrency would halve the 2x rate); head first, tail second,
            # so the GpSimd tree only ever overlaps the 1x DVE reduce ----
            tmpD = tpool.tile([128, E, 128], BF16, tag="tmpD")
            nc.vector.tensor_tensor(
                tmpD[:, 0:EDR, :], dept[:, 0:EDR, :],
                attnb[:].unsqueeze(1).broadcast_to([128, EDR, 128]),
                op=OP.mult)
            nc.vector.tensor_tensor(
                tmpD[:, EDR:E, :], dept[:, EDR:E, :],
                attnb[:].unsqueeze(1).broadcast_to([128, E - EDR, 128]),
                op=OP.mult)
            tDs = tpool.tile([128, E - EDR, 64], BF16, tag="tDs")
            dvb = spool.tile([128, E], BF16, tag="dvb")
            nc.gpsimd.tensor_tensor(tDs[:, :, :], tmpD[:, EDR:E, 0:64],
                                    tmpD[:, EDR:E, 64:128], op=OP.add)
            nc.vector.tensor_reduce(dvb[:, 0:EDR], tmpD[:, 0:EDR, :],
                                    axis=AX.X, op=OP.add)
            nc.gpsimd.tensor_tensor(tmpD[:, EDR:E, 0:32], tDs[:, :, 0:32],
                                    tDs[:, :, 32:64], op=OP.add)
            nc.gpsimd.tensor_tensor(tDs[:, :, 0:16], tmpD[:, EDR:E, 0:16],
                                    tmpD[:, EDR:E, 16:32], op=OP.add)
            nc.gpsimd.tensor_tensor(tmpD[:, EDR:E, 0:8], tDs[:, :, 0:8],
                                    tDs[:, :, 8:16], op=OP.add)
            nc.gpsimd.tensor_tensor(tDs[:, :, 0:4], tmpD[:, EDR:E, 0:4],
                                    tmpD[:, EDR:E, 4:8], op=OP.add)
            nc.gpsimd.tensor_tensor(tmpD[:, EDR:E, 0:2], tDs[:, :, 0:2],
                                    tDs[:, :, 2:4], op=OP.add)
            nc.gpsimd.tensor_tensor(dvb[:, EDR:E], tmpD[:, EDR:E, 0:1],
                                    tmpD[:, EDR:E, 1:2], op=OP.add)
            if b == 0:
                dump("d_dvec", dvb[:])
            st.update(dvb=dvb)

        def finish(b, st):
            p_t = st["p_t"]
            nc.tensor.transpose(st["ptb"][0:E, 128:256], st["dvb"][:],
                                id_bf[:])
            dT = spool.tile([E, 128], BF16, tag="dT")
            nc.scalar.copy(dT[:], st["ptb"][0:E, 128:256])
            for ns in (slice(0, 512), slice(512, H)):
                nc.tensor.matmul(p_t[:, ns], dT[:], g1[:, ns],
                                 start=False, stop=True)
            nc.tensor.matmul(p_t[:, 1022:1023], vrow4[0:1, b, :],
                             ones_b[0:1, 0:1], start=True, stop=True)
            if b == 0:
                dump("d_upd", p_t[:, 1022:1023])
            outt = opool.tile([128, H], F32, tag="outt")
            nc.vector.scalar_tensor_tensor(
                outt[:], p_t[:, 0:H], p_t[:, 1022:1023], st["bertS"][:],
                op0=OP.mult, op1=OP.add)
            # store via GpSimd SWDGE: its descriptors spread across all 16
            # DMA engines, while HWDGE (sync/scalar) dma_starts serialize on
            # ONE engine (~25GB/s) -- that cost a ~20us kernel-tail backlog
            nc.sync.dma_start(out[b, 1:44, :], outt[0:43, :])
            nc.scalar.dma_start(out[b, 44:86, :], outt[43:85, :])
            nc.gpsimd.dma_start(out[b, 86:128, :], outt[85:127, :])
            nc.sync.dma_start(out[b, 0:1, :], outt[127:128, :])

        sts = {0: st0}
        for b in range(PB):
            # prefetch first so dep(b+1) is never queued behind stores
            if b + 1 < PB:
                sts[b + 1] = prefetch(b + 1)
            if b == 0:
                load_big_weights()
            pe_front(b, sts[b])
            score_phase(b, sts[b])
            if b >= 1:
                dphase(b - 1, sts[b - 1], EDR)
            softmax_tail(b, sts[b])
            if b >= 1:
                finish(b - 1, sts.pop(b - 1))
        # final batch: DVE-heavy reduce split (DVE is otherwise idle in the
        # tail, and the GpSimd tree would pace the whole epilogue)
        dphase(PB - 1, sts[PB - 1], cfg["edrl"])
        finish(PB - 1, sts.pop(PB - 1))


def _get_nc():
    if "nc" not in _CACHED:
        _CACHED["nc"] = _build(debug=bool(_CACHED.get("debug")))
    return _CACHED["nc"]


def _chunkT(w):
    """W [rows, K] -> W^T chunk-major [128, K//128, rows] (lhsT layout)."""
    rows, k = w.shape
    return np.ascontiguousarray(
        w.T.reshape(k // 128, 128, rows).transpose(1, 0, 2))


def _prep_in_maps(bert_hidden_states, dep_type_adj, deprel_adj,
                  asp_start, asp_end, Wz, bz, wa, ba, Wf, Wh):
    bf = ml_dtypes.bfloat16
    bert = np.ascontiguousarray(np.asarray(bert_hidden_states, np.float32))
    wa_f = np.asarray(wa, np.float32)
    wa_i, wa_j, wae_f = wa_f[:H], wa_f[H:2 * H], wa_f[2 * H:]
    wae_safe = np.where(wae_f == 0.0, 1.0, wae_f)
    # dep': wa_e folded in, transposed to [b, i, e, j]
    depW = np.asarray(dep_type_adj, np.float32) * wae_f[None, None, None, :]
    dept = np.ascontiguousarray(depW.transpose(0, 1, 3, 2)).astype(bf)
    adjn = np.asarray(deprel_adj) > 0
    madd = np.where(adjn, np.float32(0.0), np.float32(MASK_NEG))
    maddT = np.ascontiguousarray(madd.transpose(0, 2, 1)).astype(bf)
    # bertS^T chunk-major per batch: rows shifted by one (the z-roll)
    bs = np.ascontiguousarray(np.roll(bert, -1, axis=1))
    bertsT = np.ascontiguousarray(
        bs.transpose(0, 2, 1).reshape(B, KC, 128, L).transpose(0, 2, 1, 3)
    ).astype(bf)
    pos = np.arange(L, dtype=np.float32)
    s_ = np.asarray(asp_start).astype(np.float32)[:, None]
    e_ = np.asarray(asp_end).astype(np.float32)[:, None]
    vrow_full = (((pos[None, :] >= s_) & (pos[None, :] <= e_))
                 & adjn.any(-1)).astype(bf)

    Wz = np.asarray(Wz, np.float32)
    bz_f = np.asarray(bz, np.float32)
    ba_f = np.float32(np.asarray(ba, np.float32))
    Wf = np.asarray(Wf, np.float32)
    Wh = np.asarray(Wh, np.float32)
    WfZ, WfE = Wf[:, :H], Wf[:, H:]
    WhN, WhZ = Wh[:, :H], Wh[:, H:]
    G0 = WhN @ WfZ
    g0wT = _chunkT(G0 @ Wz).astype(bf)
    m1T = _chunkT(WhZ @ Wz - np.eye(H, dtype=np.float32)).astype(bf)
    g1 = np.ascontiguousarray(
        (WhN @ WfE).T / wae_safe[:, None]).astype(bf)
    u2 = np.stack([Wz.T @ wa_i, Wz.T @ wa_j], axis=0)  # [2, H]
    u2T = _chunkT(u2).astype(bf)
    brow = (WhZ @ bz_f + G0 @ bz_f)[None, :].astype(bf)
    bab = np.float32(ba_f + wa_i @ bz_f + wa_j @ bz_f).reshape(1, 1)

    in_maps = []
    for c in range(NCORES):
        s = slice(c * PB, (c + 1) * PB)
        in_maps.append(dict(
            berts=bs[s], bertsT=np.ascontiguousarray(bertsT[s]),
            dept=dept[s], maddT=maddT[s],
            vrow=np.ascontiguousarray(vrow_full[s][None, :, :]),
            g0wT=g0wT, m1T=m1T, g1=g1, u2T=u2T,
            browt=brow, bat=bab,
        ))
    return in_maps


def kernel(bert_hidden_states, dep_type_adj, deprel_adj, asp_start, asp_end,
           Wz, bz, wa, ba, Wf, Wh):
    from concourse.bass_utils import run_bass_kernel_spmd

    in_maps = _prep_in_maps(bert_hidden_states, dep_type_adj, deprel_adj,
                            asp_start, asp_end, Wz, bz, wa, ba, Wf, Wh)
    nc = _get_nc()
    res = run_bass_kernel_spmd(nc, in_maps, core_ids=list(range(NCORES)),
                               trace=bool(_CACHED.get("trace")),
                               tmpdir=_CACHED.get("trace_tmpdir"))
    _CACHED["last_results"] = res
    outs = [res.results[c]["out"] for c in range(NCORES)]
    return np.concatenate(outs, axis=0).astype(np.float32)
